# revision 4
# baseline (speedup 1.0000x reference)
"""EpsGINConv TRN2 kernel v2.1: dma_gather + one-hot matmul, bf16 data path.

Structure (same as v2):
  - Gather via GPSIMD dma_gather ucode with x split into lo (<32767) / hi
    halves to fit int16 indices. Edge streams are compacted per window:
    blocks of 128 edges may span window boundaries; such blocks get one
    matmul per touched window with a masked one-hot.
  - Aggregation: psum_hT[f, dst] += Xg_block.T @ OneHot(dstl) — produces h.T
    directly. One-hot built on DVE via tensor_scalar(iota, dstl, is_equal).
  - Self term (1+eps)x added from a host-staged transposed rank-ordered x.

v2.1: gathered x, iota, and one-hots are bf16 (PSUM accumulation stays f32;
MLP stays f32). dma_gather is descriptor-generation-bound (~8.5ns/idx
regardless of dtype), but bf16 halves gather SBUF/DMA bytes and makes the
aggregation matmuls bf16 (PE LDWEIGHTS ~119ns vs ~166ns fp32), which
tightens overlap under the gather stream: 714us -> 690us. rel l2 ~1.7e-3
(from bf16 x), well under the 2e-2 gate.
"""
import sys

import numpy as np

if "/opt/trn_rl_repo" not in sys.path:
    sys.path.insert(0, "/opt/trn_rl_repo")

import concourse.bass as bass
import concourse.bacc as bacc
import concourse.tile as tile
import concourse.mybir as mybir
from concourse.bass_utils import run_bass_kernel_spmd
from concourse import library_config

P = 128
N_NODES = 50000
D = 128
N_CORES = 8
NPC = N_NODES // N_CORES
NW = (NPC + P - 1) // P
NPAD = NW * P
LO_N = 32767  # x rows [0, 32767) -> lo; row 32767 of x_lo = zeros
HI_ROWS = N_NODES - LO_N + 1  # 17234 incl zero row at 17233
IPC = 1024  # idx per dma_gather call (8 blocks); 8.8ns/idx vs 9.3 at 512, crash boundary is >=1920

F32 = mybir.dt.float32
BF16 = mybir.dt.bfloat16
I16 = mybir.dt.int16
I32 = mybir.dt.int32


def _wrap_idx(calls):
    """calls: list of 1-D int16 arrays (each len%128==0). Returns [128, cols]
    int16 in dma_gather layout: pos j of call -> partition j%16, col j//16,
    replicated 8x down partition groups; plus per-call col offsets."""
    cols = sum(len(a) // 16 for a in calls)
    lay = np.zeros((P, cols), dtype=np.int16)
    offs = []
    o = 0
    for a in calls:
        w = len(a) // 16
        blk = a.reshape(w, 16).T  # [16, w]
        for r in range(8):
            lay[r * 16 : (r + 1) * 16, o : o + w] = blk
        offs.append((o, w, len(a)))
        o += w
    return lay, offs


def _prep_host2(edge_index):
    src = edge_index[0].astype(np.int64)
    dst = edge_index[1].astype(np.int64)

    deg_all = np.bincount(dst, minlength=N_NODES)
    gorder = np.argsort(-deg_all, kind="stable")
    core_of = np.empty(N_NODES, dtype=np.int64)
    crank_of = np.empty(N_NODES, dtype=np.int64)
    grank = np.arange(N_NODES)
    core_of[gorder] = grank % N_CORES
    crank_of[gorder] = grank // N_CORES
    node_at = np.full((N_CORES, NPAD), -1, dtype=np.int64)
    node_at[core_of[gorder], crank_of[gorder]] = gorder

    e_core = core_of[dst]
    e_rank = crank_of[dst]

    per_core = []
    for c in range(N_CORES):
        m = e_core == c
        r_e = e_rank[m]
        s_e = src[m]
        o = np.argsort(r_e, kind="stable")
        per_core.append((r_e[o], s_e[o]))

    # build per-core lo/hi streams; shapes must match across cores -> compute
    # per-stream lengths per core, pad all cores' streams to the max.
    streams = []  # per core: dict(lo=(idx,win,dstl), hi=...)
    for c in range(N_CORES):
        r_s, s_s = per_core[c]
        lo_m = s_s < LO_N
        d = {}
        for name, mm, base in (("lo", lo_m, 0), ("hi", ~lo_m, LO_N)):
            rr = r_s[mm]
            ss = s_s[mm] - base
            d[name] = (ss, rr // P, rr % P)  # idx, window, dstl
        streams.append(d)

    lens = {
        name: max(len(streams[c][name][0]) for c in range(N_CORES))
        for name in ("lo", "hi")
    }
    # pad to common length, then to x128
    out = {}
    for name, zrow in (("lo", LO_N), ("hi", HI_ROWS - 1)):
        L = -(-lens[name] // P) * P
        idx_all = np.full((N_CORES, L), zrow, dtype=np.int64)
        win_all = np.full((N_CORES, L), -1, dtype=np.int64)
        dstl_all = np.full((N_CORES, L), -1, dtype=np.int64)
        for c in range(N_CORES):
            ii, ww, dd = streams[c][name]
            n = len(ii)
            idx_all[c, :n] = ii
            win_all[c, :n] = ww
            dstl_all[c, :n] = dd
        out[name] = (idx_all.astype(np.int16), win_all, dstl_all, L)

    # pairs: per stream, per block, touched windows (order by core-0's... must
    # be identical across cores! windows touched by block b can differ per
    # core. Use the UNION across cores so the program is common.
    pair_list = []  # (stream, block, window) in program order grouping by window
    pairs_by_window = [[] for _ in range(NW)]
    dstl_cols = []  # list of (stream, block, window) -> one column per pair
    for sname in ("lo", "hi"):
        idx_all, win_all, dstl_all, L = out[sname]
        nb = L // P
        for b in range(nb):
            ws = set()
            for c in range(N_CORES):
                ws |= set(np.unique(win_all[:, b * P : (b + 1) * P]).tolist())
            ws.discard(-1)
            for w in sorted(ws):
                pid = len(dstl_cols)
                dstl_cols.append((sname, b, w))
                pairs_by_window[w].append((sname, b, pid))

    n_pairs = len(dstl_cols)
    dstl_tab = np.full((N_CORES, P, n_pairs), -1.0, dtype=np.float32)
    for pid, (sname, b, w) in enumerate(dstl_cols):
        _, win_all, dstl_all, _ = out[sname]
        for c in range(N_CORES):
            wv = win_all[c, b * P : (b + 1) * P]
            dv = dstl_all[c, b * P : (b + 1) * P]
            sel = wv == w
            dstl_tab[c, sel, pid] = dv[sel]

    # calls: groups of IPC idx per stream
    call_info = {}
    idx_lay = {}
    for sname in ("lo", "hi"):
        idx_all, _, _, L = out[sname]
        calls = []
        spans = []
        o = 0
        while o < L:
            n = min(IPC, L - o)
            spans.append((o // P, n // P, n))  # first block, nblocks, nidx
            o += n
        call_info[sname] = spans
        lays = []
        for c in range(N_CORES):
            arrs = [idx_all[c, s * P : s * P + n] for (s, _, n) in spans]
            lay, offs = _wrap_idx(arrs)
            lays.append(lay)
            call_info[sname + "_offs"] = offs
        idx_lay[sname] = np.stack(lays)

    return out, pairs_by_window, dstl_tab, call_info, idx_lay, node_at, n_pairs


def _build_program2(pairs_by_window, call_info, n_pairs, lo_cols, hi_cols, eps_val):
    nc = bacc.Bacc("TRN2", target_bir_lowering=False, debug=False, num_devices=N_CORES,
                   num_swdge_queues=4)
    xlo_d = nc.dram_tensor("x_lo", [LO_N + 1, D], BF16, kind="ExternalInput").ap()
    xhi_d = nc.dram_tensor("x_hi", [HI_ROWS, D], BF16, kind="ExternalInput").ap()
    xt_d = nc.dram_tensor("xt_rank", [P, NPAD], F32, kind="ExternalInput").ap()
    ilo_d = nc.dram_tensor("idx_lo", [P, lo_cols], I16, kind="ExternalInput").ap()
    ihi_d = nc.dram_tensor("idx_hi", [P, hi_cols], I16, kind="ExternalInput").ap()
    dstl_d = nc.dram_tensor("dstl", [P, n_pairs], F32, kind="ExternalInput").ap()
    iota_d = nc.dram_tensor("iota", [P, P], BF16, kind="ExternalInput").ap()
    w1_d = nc.dram_tensor("w1", [D, D], F32, kind="ExternalInput").ap()
    w2_d = nc.dram_tensor("w2", [D, D], F32, kind="ExternalInput").ap()
    b1_d = nc.dram_tensor("b1c", [P, 1], F32, kind="ExternalInput").ap()
    b2_d = nc.dram_tensor("b2b", [P, D], F32, kind="ExternalInput").ap()
    out_d = nc.dram_tensor("out", [NPAD, D], F32, kind="ExternalOutput").ap()

    Relu = mybir.ActivationFunctionType.Relu
    IsEq = mybir.AluOpType.is_equal
    srcs = {"lo": xlo_d, "hi": xhi_d}
    idxs_d = {"lo": ilo_d, "hi": ihi_d}

    with tile.TileContext(nc) as tc:
        with (
            tc.tile_pool(name="const", bufs=1) as cp,
            tc.tile_pool(name="glo", bufs=8) as glo_p,
            tc.tile_pool(name="ghi", bufs=8) as ghi_p,
            tc.tile_pool(name="oh", bufs=16) as ohp,
            tc.tile_pool(name="hbuf", bufs=4) as hpool,
            tc.tile_pool(name="obuf", bufs=3) as opool,
            tc.tile_pool(name="pht", bufs=2, space="PSUM") as phtp,
            tc.tile_pool(name="pz", bufs=2, space="PSUM") as pzp,
            tc.tile_pool(name="po", bufs=2, space="PSUM") as pop,
        ):
            with tc.tile_critical():
                nc.gpsimd.load_library(library_config.mlp)
            ilo_t = cp.tile([P, lo_cols], I16)
            ihi_t = cp.tile([P, hi_cols], I16)
            NCH = 3
            for lo, hi in [(i * lo_cols // NCH, (i + 1) * lo_cols // NCH) for i in range(NCH)]:
                nc.sync.dma_start(ilo_t[:, lo:hi], ilo_d[:, lo:hi])
            for lo, hi in [(i * hi_cols // NCH, (i + 1) * hi_cols // NCH) for i in range(NCH)]:
                nc.sync.dma_start(ihi_t[:, lo:hi], ihi_d[:, lo:hi])
            dstl_t = cp.tile([P, n_pairs], F32)
            nc.sync.dma_start(dstl_t[:], dstl_d[:])
            iota_t = cp.tile([P, P], BF16)
            nc.sync.dma_start(iota_t[:], iota_d[:])
            xt_t = cp.tile([P, NPAD], F32)
            nc.sync.dma_start(xt_t[:], xt_d[:])
            w1_t = cp.tile([D, D], F32)
            nc.sync.dma_start(w1_t[:], w1_d[:])
            w2_t = cp.tile([D, D], F32)
            nc.sync.dma_start(w2_t[:], w2_d[:])
            b1_t = cp.tile([P, 1], F32)
            nc.sync.dma_start(b1_t[:], b1_d[:])
            b2_t = cp.tile([P, D], F32)
            nc.sync.dma_start(b2_t[:], b2_d[:])

            gp = {"lo": glo_p, "hi": ghi_p}
            g_tiles = {"lo": {}, "hi": {}}
            issued = {"lo": set(), "hi": set()}
            qctr = [0]

            def ensure_call(sname, ci):
                if ci in issued[sname]:
                    return
                issued[sname].add(ci)
                b0, nb, nidx = call_info[sname][ci]
                off, wcols, _ = call_info[sname + "_offs"][ci]
                g = gp[sname].tile([P, nb, D], BF16, tag="g")
                nc.gpsimd.dma_gather(
                    g[:],
                    srcs[sname][:],
                    idxs_d and (ilo_t if sname == "lo" else ihi_t)[:, off : off + wcols],
                    nidx,
                    nidx,
                    D,
                    queue_num=qctr[0] % 4,
                )
                qctr[0] += 1
                g_tiles[sname][ci] = g

            for w in range(NW):
                pairs = pairs_by_window[w]
                psum_ht = phtp.tile([P, D], F32, tag="pht")
                for i, (sname, b, pid) in enumerate(pairs):
                    ci = b // (IPC // P)
                    ensure_call(sname, ci)
                    b0, nb, _ = call_info[sname][ci]
                    g = g_tiles[sname][ci]
                    oh = ohp.tile([P, P], BF16, tag="oh")
                    nc.vector.tensor_scalar(
                        oh[:], iota_t[:], dstl_t[:, pid : pid + 1], None, IsEq
                    )
                    nc.tensor.matmul(
                        psum_ht[:],
                        lhsT=g[:, b - b0, :],
                        rhs=oh[:],
                        start=(i == 0),
                        stop=(i == len(pairs) - 1),
                    )
                ht_sb = hpool.tile([P, D], F32, tag="ht")
                xt_win = xt_t[:, w * P : (w + 1) * P]
                if not pairs:
                    if eps_val != 0.0:
                        nc.scalar.mul(ht_sb[:], xt_win, float(1.0 + eps_val))
                    else:
                        nc.vector.tensor_copy(ht_sb[:], xt_win)
                elif eps_val != 0.0:
                    xs_t = hpool.tile([P, D], F32, tag="xs")
                    nc.scalar.mul(xs_t[:], xt_win, float(1.0 + eps_val))
                    nc.vector.tensor_add(ht_sb[:], xs_t[:], psum_ht[:])
                else:
                    nc.vector.tensor_add(ht_sb[:], xt_win, psum_ht[:])
                psum_z = pzp.tile([P, D], F32, tag="pz")
                nc.tensor.matmul(psum_z[:], lhsT=w1_t[:], rhs=ht_sb[:], start=True, stop=True)
                zt_sb = hpool.tile([P, D], F32, tag="zt")
                nc.scalar.activation(zt_sb[:], psum_z[:], Relu, bias=b1_t[:, :1])
                psum_o = pop.tile([P, D], F32, tag="po")
                nc.tensor.matmul(psum_o[:], lhsT=zt_sb[:], rhs=w2_t[:], start=True, stop=True)
                o_sb = opool.tile([P, D], F32, tag="o")
                nc.vector.tensor_add(o_sb[:], psum_o[:], b2_t[:])
                o2_sb = opool.tile([P, D], F32, tag="o2")
                nc.scalar.activation(o2_sb[:], o_sb[:], Relu)
                nc.sync.dma_start(out_d[w * P : (w + 1) * P, :], o2_sb[:])
    nc.compile()
    return nc


def kernel(x, edge_index, W1, b1, W2, b2, eps):
    x = np.ascontiguousarray(np.asarray(x, dtype=np.float32))
    W1 = np.asarray(W1, dtype=np.float32)
    W2 = np.asarray(W2, dtype=np.float32)
    b1 = np.asarray(b1, dtype=np.float32)
    b2 = np.asarray(b2, dtype=np.float32)
    eps_val = float(np.asarray(eps))

    (out_s, pairs_by_window, dstl_tab, call_info, idx_lay, node_at, n_pairs) = _prep_host2(
        np.asarray(edge_index)
    )
    lo_cols = idx_lay["lo"].shape[2]
    hi_cols = idx_lay["hi"].shape[2]
    nc = _build_program2(pairs_by_window, call_info, n_pairs, lo_cols, hi_cols, eps_val)

    import ml_dtypes
    x_lo = np.ascontiguousarray(
        np.concatenate([x[:LO_N], np.zeros((1, D), np.float32)], axis=0).astype(ml_dtypes.bfloat16))
    x_hi = np.ascontiguousarray(
        np.concatenate([x[LO_N:], np.zeros((1, D), np.float32)], axis=0).astype(ml_dtypes.bfloat16))
    b1c = np.ascontiguousarray(b1.reshape(P, 1))
    b2b = np.ascontiguousarray(np.tile(b2.reshape(1, D), (P, 1)))
    iota = np.ascontiguousarray(np.tile(np.arange(P, dtype=np.float32), (P, 1)).astype(ml_dtypes.bfloat16))

    in_maps = []
    for c in range(N_CORES):
        ids = node_at[c].copy()
        zpad = ids < 0
        ids[zpad] = 0
        xt = x[ids].T.copy()
        xt[:, zpad] = 0.0
        in_maps.append(
            {
                "x_lo": x_lo,
                "x_hi": x_hi,
                "xt_rank": np.ascontiguousarray(xt),
                "idx_lo": np.ascontiguousarray(idx_lay["lo"][c]),
                "idx_hi": np.ascontiguousarray(idx_lay["hi"][c]),
                "dstl": np.ascontiguousarray(dstl_tab[c]),
                "iota": iota,
                "w1": W1,
                "w2": W2,
                "b1c": b1c,
                "b2b": b2b,
            }
        )
    res = run_bass_kernel_spmd(nc, in_maps, list(range(N_CORES)))

    out = np.empty((N_NODES, D), dtype=np.float32)
    for c in range(N_CORES):
        rows = np.asarray(res.results[c]["out"])
        ids = node_at[c]
        valid = ids >= 0
        out[ids[valid]] = rows[valid]
    kernel.last_results = res
    return out



# revision 5
# speedup vs baseline: 1.2014x; 1.2014x over previous
"""EpsGINConv TRN2 kernel v2.1: dma_gather + one-hot matmul, bf16 data path.

Structure (same as v2):
  - Gather via GPSIMD dma_gather ucode with x split into lo (<32767) / hi
    halves to fit int16 indices. Edge streams are compacted per window:
    blocks of 128 edges may span window boundaries; such blocks get one
    matmul per touched window with a masked one-hot.
  - Aggregation: psum_hT[f, dst] += Xg_block.T @ OneHot(dstl) — produces h.T
    directly. One-hot built on DVE via tensor_scalar(iota, dstl, is_equal).
  - Self term (1+eps)x added from a host-staged transposed rank-ordered x.

v2.1: gathered x, iota, and one-hots are bf16 (PSUM accumulation stays f32;
MLP stays f32). dma_gather is descriptor-generation-bound (~8.5ns/idx
regardless of dtype), but bf16 halves gather SBUF/DMA bytes and makes the
aggregation matmuls bf16 (PE LDWEIGHTS ~119ns vs ~166ns fp32), which
tightens overlap under the gather stream: 714us -> 690us. rel l2 ~1.7e-3
(from bf16 x), well under the 2e-2 gate.
"""
import sys

import numpy as np

if "/opt/trn_rl_repo" not in sys.path:
    sys.path.insert(0, "/opt/trn_rl_repo")

import concourse.bass as bass
import concourse.bacc as bacc
import concourse.tile as tile
import concourse.mybir as mybir
from concourse.bass_utils import run_bass_kernel_spmd
from concourse import library_config

P = 128
N_NODES = 50000
D = 128
N_CORES = 8
NPC = N_NODES // N_CORES
NW = (NPC + P - 1) // P
NPAD = NW * P
LO_N = 32767  # x rows [0, 32767) -> lo; row 32767 of x_lo = zeros
HI_ROWS = N_NODES - LO_N + 1  # 17234 incl zero row at 17233
IPC = 1024  # idx per dma_gather call (8 blocks); 8.8ns/idx vs 9.3 at 512, crash boundary is >=1920

F32 = mybir.dt.float32
BF16 = mybir.dt.bfloat16
I16 = mybir.dt.int16
I32 = mybir.dt.int32


def _wrap_idx(calls):
    """calls: list of 1-D int16 arrays (each len%128==0). Returns [128, cols]
    int16 in dma_gather layout: pos j of call -> partition j%16, col j//16,
    replicated 8x down partition groups; plus per-call col offsets."""
    cols = sum(len(a) // 16 for a in calls)
    lay = np.zeros((P, cols), dtype=np.int16)
    offs = []
    o = 0
    for a in calls:
        w = len(a) // 16
        blk = a.reshape(w, 16).T  # [16, w]
        for r in range(8):
            lay[r * 16 : (r + 1) * 16, o : o + w] = blk
        offs.append((o, w, len(a)))
        o += w
    return lay, offs


def _prep_host2(edge_index):
    src = edge_index[0].astype(np.int64)
    dst = edge_index[1].astype(np.int64)

    deg_all = np.bincount(dst, minlength=N_NODES)
    gorder = np.argsort(-deg_all, kind="stable")
    core_of = np.empty(N_NODES, dtype=np.int64)
    crank_of = np.empty(N_NODES, dtype=np.int64)
    grank = np.arange(N_NODES)
    core_of[gorder] = grank % N_CORES
    crank_of[gorder] = grank // N_CORES
    node_at = np.full((N_CORES, NPAD), -1, dtype=np.int64)
    node_at[core_of[gorder], crank_of[gorder]] = gorder

    e_core = core_of[dst]
    e_rank = crank_of[dst]

    per_core = []
    for c in range(N_CORES):
        m = e_core == c
        r_e = e_rank[m]
        s_e = src[m]
        o = np.argsort(r_e, kind="stable")
        per_core.append((r_e[o], s_e[o]))

    # build per-core lo/hi streams; shapes must match across cores -> compute
    # per-stream lengths per core, pad all cores' streams to the max.
    streams = []  # per core: dict(lo=(idx,win,dstl), hi=...)
    for c in range(N_CORES):
        r_s, s_s = per_core[c]
        lo_m = s_s < LO_N
        d = {}
        for name, mm, base in (("lo", lo_m, 0), ("hi", ~lo_m, LO_N)):
            rr = r_s[mm]
            ss = s_s[mm] - base
            d[name] = (ss, rr // P, rr % P)  # idx, window, dstl
        streams.append(d)

    lens = {
        name: max(len(streams[c][name][0]) for c in range(N_CORES))
        for name in ("lo", "hi")
    }
    # pad to common length, then to x128
    out = {}
    for name, zrow in (("lo", LO_N), ("hi", HI_ROWS - 1)):
        L = -(-lens[name] // P) * P
        idx_all = np.full((N_CORES, L), zrow, dtype=np.int64)
        win_all = np.full((N_CORES, L), -1, dtype=np.int64)
        dstl_all = np.full((N_CORES, L), -1, dtype=np.int64)
        for c in range(N_CORES):
            ii, ww, dd = streams[c][name]
            n = len(ii)
            idx_all[c, :n] = ii
            win_all[c, :n] = ww
            dstl_all[c, :n] = dd
        out[name] = (idx_all.astype(np.int16), win_all, dstl_all, L)

    # pairs: per stream, per block, touched windows (order by core-0's... must
    # be identical across cores! windows touched by block b can differ per
    # core. Use the UNION across cores so the program is common.
    pair_list = []  # (stream, block, window) in program order grouping by window
    pairs_by_window = [[] for _ in range(NW)]
    dstl_cols = []  # list of (stream, block, window) -> one column per pair
    for sname in ("lo", "hi"):
        idx_all, win_all, dstl_all, L = out[sname]
        nb = L // P
        for b in range(nb):
            ws = set()
            for c in range(N_CORES):
                ws |= set(np.unique(win_all[:, b * P : (b + 1) * P]).tolist())
            ws.discard(-1)
            for w in sorted(ws):
                pid = len(dstl_cols)
                dstl_cols.append((sname, b, w))
                pairs_by_window[w].append((sname, b, pid))

    n_pairs = len(dstl_cols)
    dstl_tab = np.full((N_CORES, P, n_pairs), -1.0, dtype=np.float32)
    for pid, (sname, b, w) in enumerate(dstl_cols):
        _, win_all, dstl_all, _ = out[sname]
        for c in range(N_CORES):
            wv = win_all[c, b * P : (b + 1) * P]
            dv = dstl_all[c, b * P : (b + 1) * P]
            sel = wv == w
            dstl_tab[c, sel, pid] = dv[sel]

    # calls: groups of IPC idx per stream
    call_info = {}
    idx_lay = {}
    for sname in ("lo", "hi"):
        idx_all, _, _, L = out[sname]
        calls = []
        spans = []
        o = 0
        while o < L:
            n = min(IPC, L - o)
            spans.append((o // P, n // P, n))  # first block, nblocks, nidx
            o += n
        call_info[sname] = spans
        lays = []
        for c in range(N_CORES):
            arrs = [idx_all[c, s * P : s * P + n] for (s, _, n) in spans]
            lay, offs = _wrap_idx(arrs)
            lays.append(lay)
            call_info[sname + "_offs"] = offs
        idx_lay[sname] = np.stack(lays)

    return out, pairs_by_window, dstl_tab, call_info, idx_lay, node_at, n_pairs


def _build_program2(pairs_by_window, call_info, n_pairs, lo_cols, hi_cols, eps_val):
    nc = bacc.Bacc("TRN2", target_bir_lowering=False, debug=False, num_devices=N_CORES,
                   num_swdge_queues=4)
    xlo_d = nc.dram_tensor("x_lo", [LO_N + 1, D], BF16, kind="ExternalInput").ap()
    xhi_d = nc.dram_tensor("x_hi", [HI_ROWS, D], BF16, kind="ExternalInput").ap()
    xt_d = nc.dram_tensor("xt_rank", [P, NPAD], F32, kind="ExternalInput").ap()
    ilo_d = nc.dram_tensor("idx_lo", [P, lo_cols], I16, kind="ExternalInput").ap()
    ihi_d = nc.dram_tensor("idx_hi", [P, hi_cols], I16, kind="ExternalInput").ap()
    dstl_d = nc.dram_tensor("dstl", [P, n_pairs], F32, kind="ExternalInput").ap()
    iota_d = nc.dram_tensor("iota", [P, P], BF16, kind="ExternalInput").ap()
    w1_d = nc.dram_tensor("w1", [D, D], F32, kind="ExternalInput").ap()
    w2_d = nc.dram_tensor("w2", [D, D], F32, kind="ExternalInput").ap()
    b1_d = nc.dram_tensor("b1c", [P, 1], F32, kind="ExternalInput").ap()
    b2_d = nc.dram_tensor("b2b", [P, D], F32, kind="ExternalInput").ap()
    out_d = nc.dram_tensor("out", [NPAD, D], F32, kind="ExternalOutput").ap()

    Relu = mybir.ActivationFunctionType.Relu
    IsEq = mybir.AluOpType.is_equal
    srcs = {"lo": xlo_d, "hi": xhi_d}
    idxs_d = {"lo": ilo_d, "hi": ihi_d}

    with tile.TileContext(nc) as tc:
        with (
            tc.tile_pool(name="const", bufs=1) as cp,
            tc.tile_pool(name="glo", bufs=16) as glo_p,
            tc.tile_pool(name="ghi", bufs=16) as ghi_p,
            tc.tile_pool(name="oh", bufs=40) as ohp,
            tc.tile_pool(name="hbuf", bufs=4) as hpool,
            tc.tile_pool(name="obuf", bufs=3) as opool,
            tc.tile_pool(name="pht", bufs=2, space="PSUM") as phtp,
            tc.tile_pool(name="pz", bufs=2, space="PSUM") as pzp,
            tc.tile_pool(name="po", bufs=2, space="PSUM") as pop,
        ):
            with tc.tile_critical():
                nc.gpsimd.load_library(library_config.mlp)
            ilo_t = cp.tile([P, lo_cols], I16)
            ihi_t = cp.tile([P, hi_cols], I16)
            NCH = 3
            for lo, hi in [(i * lo_cols // NCH, (i + 1) * lo_cols // NCH) for i in range(NCH)]:
                nc.sync.dma_start(ilo_t[:, lo:hi], ilo_d[:, lo:hi])
            for lo, hi in [(i * hi_cols // NCH, (i + 1) * hi_cols // NCH) for i in range(NCH)]:
                nc.sync.dma_start(ihi_t[:, lo:hi], ihi_d[:, lo:hi])
            dstl_t = cp.tile([P, n_pairs], F32)
            nc.sync.dma_start(dstl_t[:], dstl_d[:])
            iota_t = cp.tile([P, P], BF16)
            nc.sync.dma_start(iota_t[:], iota_d[:])
            xt_t = cp.tile([P, NPAD], F32)
            nc.sync.dma_start(xt_t[:], xt_d[:])
            w1_t = cp.tile([D, D], F32)
            nc.sync.dma_start(w1_t[:], w1_d[:])
            w2_t = cp.tile([D, D], F32)
            nc.sync.dma_start(w2_t[:], w2_d[:])
            b1_t = cp.tile([P, 1], F32)
            nc.sync.dma_start(b1_t[:], b1_d[:])
            b2_t = cp.tile([P, D], F32)
            nc.sync.dma_start(b2_t[:], b2_d[:])

            gp = {"lo": glo_p, "hi": ghi_p}
            g_tiles = {"lo": {}, "hi": {}}
            issued = {"lo": set(), "hi": set()}
            qctr = [0]

            def ensure_call(sname, ci):
                if ci in issued[sname]:
                    return
                issued[sname].add(ci)
                b0, nb, nidx = call_info[sname][ci]
                off, wcols, _ = call_info[sname + "_offs"][ci]
                g = gp[sname].tile([P, nb, D], BF16, tag="g")
                nc.gpsimd.dma_gather(
                    g[:],
                    srcs[sname][:],
                    idxs_d and (ilo_t if sname == "lo" else ihi_t)[:, off : off + wcols],
                    nidx,
                    nidx,
                    D,
                    queue_num=qctr[0] % 4,
                )
                qctr[0] += 1
                g_tiles[sname][ci] = g

            for w in range(NW):
                pairs = pairs_by_window[w]
                psum_ht = phtp.tile([P, D], F32, tag="pht")
                for i, (sname, b, pid) in enumerate(pairs):
                    ci = b // (IPC // P)
                    ensure_call(sname, ci)
                    b0, nb, _ = call_info[sname][ci]
                    g = g_tiles[sname][ci]
                    oh = ohp.tile([P, P], BF16, tag="oh")
                    nc.vector.tensor_scalar(
                        oh[:], iota_t[:], dstl_t[:, pid : pid + 1], None, IsEq
                    )
                    nc.tensor.matmul(
                        psum_ht[:],
                        lhsT=g[:, b - b0, :],
                        rhs=oh[:],
                        start=(i == 0),
                        stop=(i == len(pairs) - 1),
                    )
                ht_sb = hpool.tile([P, D], F32, tag="ht")
                xt_win = xt_t[:, w * P : (w + 1) * P]
                if not pairs:
                    if eps_val != 0.0:
                        nc.scalar.mul(ht_sb[:], xt_win, float(1.0 + eps_val))
                    else:
                        nc.vector.tensor_copy(ht_sb[:], xt_win)
                elif eps_val != 0.0:
                    xs_t = hpool.tile([P, D], F32, tag="xs")
                    nc.scalar.mul(xs_t[:], xt_win, float(1.0 + eps_val))
                    nc.vector.tensor_add(ht_sb[:], xs_t[:], psum_ht[:])
                else:
                    nc.vector.tensor_add(ht_sb[:], xt_win, psum_ht[:])
                psum_z = pzp.tile([P, D], F32, tag="pz")
                nc.tensor.matmul(psum_z[:], lhsT=w1_t[:], rhs=ht_sb[:], start=True, stop=True)
                zt_sb = hpool.tile([P, D], F32, tag="zt")
                nc.scalar.activation(zt_sb[:], psum_z[:], Relu, bias=b1_t[:, :1])
                psum_o = pop.tile([P, D], F32, tag="po")
                nc.tensor.matmul(psum_o[:], lhsT=zt_sb[:], rhs=w2_t[:], start=True, stop=True)
                o_sb = opool.tile([P, D], F32, tag="o")
                nc.vector.tensor_add(o_sb[:], psum_o[:], b2_t[:])
                o2_sb = opool.tile([P, D], F32, tag="o2")
                nc.scalar.activation(o2_sb[:], o_sb[:], Relu)
                nc.sync.dma_start(out_d[w * P : (w + 1) * P, :], o2_sb[:])
    nc.compile()
    return nc


def kernel(x, edge_index, W1, b1, W2, b2, eps):
    x = np.ascontiguousarray(np.asarray(x, dtype=np.float32))
    W1 = np.asarray(W1, dtype=np.float32)
    W2 = np.asarray(W2, dtype=np.float32)
    b1 = np.asarray(b1, dtype=np.float32)
    b2 = np.asarray(b2, dtype=np.float32)
    eps_val = float(np.asarray(eps))

    (out_s, pairs_by_window, dstl_tab, call_info, idx_lay, node_at, n_pairs) = _prep_host2(
        np.asarray(edge_index)
    )
    lo_cols = idx_lay["lo"].shape[2]
    hi_cols = idx_lay["hi"].shape[2]
    nc = _build_program2(pairs_by_window, call_info, n_pairs, lo_cols, hi_cols, eps_val)

    import ml_dtypes
    x_lo = np.ascontiguousarray(
        np.concatenate([x[:LO_N], np.zeros((1, D), np.float32)], axis=0).astype(ml_dtypes.bfloat16))
    x_hi = np.ascontiguousarray(
        np.concatenate([x[LO_N:], np.zeros((1, D), np.float32)], axis=0).astype(ml_dtypes.bfloat16))
    b1c = np.ascontiguousarray(b1.reshape(P, 1))
    b2b = np.ascontiguousarray(np.tile(b2.reshape(1, D), (P, 1)))
    iota = np.ascontiguousarray(np.tile(np.arange(P, dtype=np.float32), (P, 1)).astype(ml_dtypes.bfloat16))

    in_maps = []
    for c in range(N_CORES):
        ids = node_at[c].copy()
        zpad = ids < 0
        ids[zpad] = 0
        xt = x[ids].T.copy()
        xt[:, zpad] = 0.0
        in_maps.append(
            {
                "x_lo": x_lo,
                "x_hi": x_hi,
                "xt_rank": np.ascontiguousarray(xt),
                "idx_lo": np.ascontiguousarray(idx_lay["lo"][c]),
                "idx_hi": np.ascontiguousarray(idx_lay["hi"][c]),
                "dstl": np.ascontiguousarray(dstl_tab[c]),
                "iota": iota,
                "w1": W1,
                "w2": W2,
                "b1c": b1c,
                "b2b": b2b,
            }
        )
    res = run_bass_kernel_spmd(nc, in_maps, list(range(N_CORES)))

    out = np.empty((N_NODES, D), dtype=np.float32)
    for c in range(N_CORES):
        rows = np.asarray(res.results[c]["out"])
        ids = node_at[c]
        valid = ids >= 0
        out[ids[valid]] = rows[valid]
    kernel.last_results = res
    return out



# revision 6
# speedup vs baseline: 1.2777x; 1.0635x over previous
"""EpsGINConv TRN2 kernel v2.1: dma_gather + one-hot matmul, bf16 data path.

Structure (same as v2):
  - Gather via GPSIMD dma_gather ucode with x split into lo (<32767) / hi
    halves to fit int16 indices. Edge streams are compacted per window:
    blocks of 128 edges may span window boundaries; such blocks get one
    matmul per touched window with a masked one-hot.
  - Aggregation: psum_hT[f, dst] += Xg_block.T @ OneHot(dstl) — produces h.T
    directly. One-hot built on DVE via tensor_scalar(iota, dstl, is_equal).
  - Self term (1+eps)x added from a host-staged transposed rank-ordered x.

v2.1: gathered x, iota, and one-hots are bf16 (PSUM accumulation stays f32;
MLP stays f32). dma_gather is descriptor-generation-bound (~8.5ns/idx
regardless of dtype), but bf16 halves gather SBUF/DMA bytes and makes the
aggregation matmuls bf16 (PE LDWEIGHTS ~119ns vs ~166ns fp32), which
tightens overlap under the gather stream: 714us -> 690us. rel l2 ~1.7e-3
(from bf16 x), well under the 2e-2 gate.
"""
import sys

import numpy as np

if "/opt/trn_rl_repo" not in sys.path:
    sys.path.insert(0, "/opt/trn_rl_repo")

import concourse.bass as bass
import concourse.bacc as bacc
import concourse.tile as tile
import concourse.mybir as mybir
from concourse.bass_utils import run_bass_kernel_spmd
from concourse import library_config

P = 128
N_NODES = 50000
D = 128
N_CORES = 8
NPC = N_NODES // N_CORES
NW = (NPC + P - 1) // P
NPAD = NW * P
LO_N = 32767  # x rows [0, 32767) -> lo; row 32767 of x_lo = zeros
HI_ROWS = N_NODES - LO_N + 1  # 17234 incl zero row at 17233
IPC = 1024  # idx per dma_gather call (8 blocks); 8.8ns/idx vs 9.3 at 512, crash boundary is >=1920

F32 = mybir.dt.float32
BF16 = mybir.dt.bfloat16
I16 = mybir.dt.int16
I32 = mybir.dt.int32


def _wrap_idx(calls):
    """calls: list of 1-D int16 arrays (each len%128==0). Returns [128, cols]
    int16 in dma_gather layout: pos j of call -> partition j%16, col j//16,
    replicated 8x down partition groups; plus per-call col offsets."""
    cols = sum(len(a) // 16 for a in calls)
    lay = np.zeros((P, cols), dtype=np.int16)
    offs = []
    o = 0
    for a in calls:
        w = len(a) // 16
        blk = a.reshape(w, 16).T  # [16, w]
        for r in range(8):
            lay[r * 16 : (r + 1) * 16, o : o + w] = blk
        offs.append((o, w, len(a)))
        o += w
    return lay, offs


def _prep_host2(edge_index):
    src = edge_index[0].astype(np.int64)
    dst = edge_index[1].astype(np.int64)

    deg_all = np.bincount(dst, minlength=N_NODES)
    gorder = np.argsort(-deg_all, kind="stable")
    core_of = np.empty(N_NODES, dtype=np.int64)
    crank_of = np.empty(N_NODES, dtype=np.int64)
    grank = np.arange(N_NODES)
    core_of[gorder] = grank % N_CORES
    crank_of[gorder] = grank // N_CORES
    node_at = np.full((N_CORES, NPAD), -1, dtype=np.int64)
    node_at[core_of[gorder], crank_of[gorder]] = gorder

    e_core = core_of[dst]
    e_rank = crank_of[dst]

    per_core = []
    for c in range(N_CORES):
        m = e_core == c
        r_e = e_rank[m]
        s_e = src[m]
        o = np.argsort(r_e, kind="stable")
        per_core.append((r_e[o], s_e[o]))

    # build per-core lo/hi streams; shapes must match across cores -> compute
    # per-stream lengths per core, pad all cores' streams to the max.
    streams = []  # per core: dict(lo=(idx,win,dstl), hi=...)
    for c in range(N_CORES):
        r_s, s_s = per_core[c]
        lo_m = s_s < LO_N
        d = {}
        for name, mm, base in (("lo", lo_m, 0), ("hi", ~lo_m, LO_N)):
            rr = r_s[mm]
            ss = s_s[mm] - base
            d[name] = (ss, rr // P, rr % P)  # idx, window, dstl
        streams.append(d)

    lens = {
        name: max(len(streams[c][name][0]) for c in range(N_CORES))
        for name in ("lo", "hi")
    }
    # pad to common length, then to x128
    out = {}
    for name, zrow in (("lo", LO_N), ("hi", HI_ROWS - 1)):
        L = -(-lens[name] // P) * P
        idx_all = np.full((N_CORES, L), zrow, dtype=np.int64)
        win_all = np.full((N_CORES, L), -1, dtype=np.int64)
        dstl_all = np.full((N_CORES, L), -1, dtype=np.int64)
        for c in range(N_CORES):
            ii, ww, dd = streams[c][name]
            n = len(ii)
            idx_all[c, :n] = ii
            win_all[c, :n] = ww
            dstl_all[c, :n] = dd
        out[name] = (idx_all.astype(np.int16), win_all, dstl_all, L)

    # pairs: per stream, per block, touched windows (order by core-0's... must
    # be identical across cores! windows touched by block b can differ per
    # core. Use the UNION across cores so the program is common.
    pair_list = []  # (stream, block, window) in program order grouping by window
    pairs_by_window = [[] for _ in range(NW)]
    dstl_cols = []  # list of (stream, block, window) -> one column per pair
    for sname in ("lo", "hi"):
        idx_all, win_all, dstl_all, L = out[sname]
        nb = L // P
        for b in range(nb):
            ws = set()
            for c in range(N_CORES):
                ws |= set(np.unique(win_all[:, b * P : (b + 1) * P]).tolist())
            ws.discard(-1)
            for w in sorted(ws):
                pid = len(dstl_cols)
                dstl_cols.append((sname, b, w))
                pairs_by_window[w].append((sname, b, pid))

    n_pairs = len(dstl_cols)
    dstl_tab = np.full((N_CORES, P, n_pairs), -1.0, dtype=np.float32)
    for pid, (sname, b, w) in enumerate(dstl_cols):
        _, win_all, dstl_all, _ = out[sname]
        for c in range(N_CORES):
            wv = win_all[c, b * P : (b + 1) * P]
            dv = dstl_all[c, b * P : (b + 1) * P]
            sel = wv == w
            dstl_tab[c, sel, pid] = dv[sel]

    # calls: groups of IPC idx per stream
    call_info = {}
    idx_lay = {}
    for sname in ("lo", "hi"):
        idx_all, _, _, L = out[sname]
        calls = []
        spans = []
        o = 0
        while o < L:
            n = min(IPC, L - o)
            spans.append((o // P, n // P, n))  # first block, nblocks, nidx
            o += n
        call_info[sname] = spans
        lays = []
        for c in range(N_CORES):
            arrs = [idx_all[c, s * P : s * P + n] for (s, _, n) in spans]
            lay, offs = _wrap_idx(arrs)
            lays.append(lay)
            call_info[sname + "_offs"] = offs
        idx_lay[sname] = np.stack(lays)

    return out, pairs_by_window, dstl_tab, call_info, idx_lay, node_at, n_pairs


def _build_program2(pairs_by_window, call_info, n_pairs, lo_cols, hi_cols, eps_val):
    nc = bacc.Bacc("TRN2", target_bir_lowering=False, debug=False, num_devices=N_CORES,
                   num_swdge_queues=4)
    xlo_d = nc.dram_tensor("x_lo", [LO_N + 1, D], BF16, kind="ExternalInput").ap()
    xhi_d = nc.dram_tensor("x_hi", [HI_ROWS, D], BF16, kind="ExternalInput").ap()
    xt_d = nc.dram_tensor("xt_rank", [P, NPAD], F32, kind="ExternalInput").ap()
    ilo_d = nc.dram_tensor("idx_lo", [P, lo_cols], I16, kind="ExternalInput").ap()
    ihi_d = nc.dram_tensor("idx_hi", [P, hi_cols], I16, kind="ExternalInput").ap()
    dstl_d = nc.dram_tensor("dstl", [P, n_pairs], F32, kind="ExternalInput").ap()
    ndstl_d = nc.dram_tensor("ndstl", [P, n_pairs], F32, kind="ExternalInput").ap()
    iota_d = nc.dram_tensor("iota", [P, P], BF16, kind="ExternalInput").ap()
    w1_d = nc.dram_tensor("w1", [D, D], F32, kind="ExternalInput").ap()
    w2_d = nc.dram_tensor("w2", [D, D], F32, kind="ExternalInput").ap()
    b1_d = nc.dram_tensor("b1c", [P, 1], F32, kind="ExternalInput").ap()
    b2_d = nc.dram_tensor("b2b", [P, D], F32, kind="ExternalInput").ap()
    out_d = nc.dram_tensor("out", [NPAD, D], F32, kind="ExternalOutput").ap()

    Relu = mybir.ActivationFunctionType.Relu
    Square = mybir.ActivationFunctionType.Square
    IsEq = mybir.AluOpType.is_equal
    srcs = {"lo": xlo_d, "hi": xhi_d}
    idxs_d = {"lo": ilo_d, "hi": ihi_d}

    with tile.TileContext(nc) as tc:
        with (
            tc.tile_pool(name="const", bufs=1) as cp,
            tc.tile_pool(name="glo", bufs=16) as glo_p,
            tc.tile_pool(name="ghi", bufs=16) as ghi_p,
            tc.tile_pool(name="oh", bufs=40) as ohp,
            tc.tile_pool(name="hbuf", bufs=4) as hpool,
            tc.tile_pool(name="obuf", bufs=3) as opool,
            tc.tile_pool(name="pht", bufs=2, space="PSUM") as phtp,
            tc.tile_pool(name="pz", bufs=2, space="PSUM") as pzp,
            tc.tile_pool(name="po", bufs=2, space="PSUM") as pop,
        ):
            with tc.tile_critical():
                nc.gpsimd.load_library(library_config.mlp)
            ilo_t = cp.tile([P, lo_cols], I16)
            ihi_t = cp.tile([P, hi_cols], I16)
            NCH = 3
            for lo, hi in [(i * lo_cols // NCH, (i + 1) * lo_cols // NCH) for i in range(NCH)]:
                nc.sync.dma_start(ilo_t[:, lo:hi], ilo_d[:, lo:hi])
            for lo, hi in [(i * hi_cols // NCH, (i + 1) * hi_cols // NCH) for i in range(NCH)]:
                nc.sync.dma_start(ihi_t[:, lo:hi], ihi_d[:, lo:hi])
            dstl_t = cp.tile([P, n_pairs], F32)
            nc.sync.dma_start(dstl_t[:], dstl_d[:])
            ndstl_t = cp.tile([P, n_pairs], F32)
            nc.sync.dma_start(ndstl_t[:], ndstl_d[:])
            iota_t = cp.tile([P, P], BF16)
            nc.sync.dma_start(iota_t[:], iota_d[:])
            xt_t = cp.tile([P, NPAD], F32)
            nc.sync.dma_start(xt_t[:], xt_d[:])
            w1_t = cp.tile([D, D], F32)
            nc.sync.dma_start(w1_t[:], w1_d[:])
            w2_t = cp.tile([D, D], F32)
            nc.sync.dma_start(w2_t[:], w2_d[:])
            b1_t = cp.tile([P, 1], F32)
            nc.sync.dma_start(b1_t[:], b1_d[:])
            b2_t = cp.tile([P, D], F32)
            nc.sync.dma_start(b2_t[:], b2_d[:])

            gp = {"lo": glo_p, "hi": ghi_p}
            g_tiles = {"lo": {}, "hi": {}}
            issued = {"lo": set(), "hi": set()}
            qctr = [0]

            def ensure_call(sname, ci):
                if ci in issued[sname]:
                    return
                issued[sname].add(ci)
                b0, nb, nidx = call_info[sname][ci]
                off, wcols, _ = call_info[sname + "_offs"][ci]
                g = gp[sname].tile([P, nb, D], BF16, tag="g")
                nc.gpsimd.dma_gather(
                    g[:],
                    srcs[sname][:],
                    idxs_d and (ilo_t if sname == "lo" else ihi_t)[:, off : off + wcols],
                    nidx,
                    nidx,
                    D,
                    queue_num=qctr[0] % 4,
                )
                qctr[0] += 1
                g_tiles[sname][ci] = g

            for w in range(NW):
                pairs = pairs_by_window[w]
                psum_ht = phtp.tile([P, D], F32, tag="pht")
                for i, (sname, b, pid) in enumerate(pairs):
                    ci = b // (IPC // P)
                    ensure_call(sname, ci)
                    b0, nb, _ = call_info[sname][ci]
                    g = g_tiles[sname][ci]
                    oh = ohp.tile([P, P], BF16, tag="oh")
                    if pid % 6 == 5:
                        t2 = ohp.tile([P, P], F32, tag="t2")
                        nc.scalar.activation(
                            t2[:], iota_t[:], Square, bias=ndstl_t[:, pid : pid + 1]
                        )
                        nc.scalar.activation(oh[:], t2[:], Relu, bias=1.0, scale=-1.0)
                    else:
                        nc.vector.tensor_scalar(
                            oh[:], iota_t[:], dstl_t[:, pid : pid + 1], None, IsEq
                        )
                    nc.tensor.matmul(
                        psum_ht[:],
                        lhsT=g[:, b - b0, :],
                        rhs=oh[:],
                        start=(i == 0),
                        stop=(i == len(pairs) - 1),
                    )
                ht_sb = hpool.tile([P, D], F32, tag="ht")
                xt_win = xt_t[:, w * P : (w + 1) * P]
                if not pairs:
                    if eps_val != 0.0:
                        nc.scalar.mul(ht_sb[:], xt_win, float(1.0 + eps_val))
                    else:
                        nc.vector.tensor_copy(ht_sb[:], xt_win)
                elif eps_val != 0.0:
                    xs_t = hpool.tile([P, D], F32, tag="xs")
                    nc.scalar.mul(xs_t[:], xt_win, float(1.0 + eps_val))
                    nc.vector.tensor_add(ht_sb[:], xs_t[:], psum_ht[:])
                else:
                    nc.vector.tensor_add(ht_sb[:], xt_win, psum_ht[:])
                psum_z = pzp.tile([P, D], F32, tag="pz")
                nc.tensor.matmul(psum_z[:], lhsT=w1_t[:], rhs=ht_sb[:], start=True, stop=True)
                zt_sb = hpool.tile([P, D], F32, tag="zt")
                nc.scalar.activation(zt_sb[:], psum_z[:], Relu, bias=b1_t[:, :1])
                psum_o = pop.tile([P, D], F32, tag="po")
                nc.tensor.matmul(psum_o[:], lhsT=zt_sb[:], rhs=w2_t[:], start=True, stop=True)
                o_sb = opool.tile([P, D], F32, tag="o")
                nc.vector.tensor_add(o_sb[:], psum_o[:], b2_t[:])
                o2_sb = opool.tile([P, D], F32, tag="o2")
                nc.scalar.activation(o2_sb[:], o_sb[:], Relu)
                nc.sync.dma_start(out_d[w * P : (w + 1) * P, :], o2_sb[:])
    nc.compile()
    return nc


def kernel(x, edge_index, W1, b1, W2, b2, eps):
    x = np.ascontiguousarray(np.asarray(x, dtype=np.float32))
    W1 = np.asarray(W1, dtype=np.float32)
    W2 = np.asarray(W2, dtype=np.float32)
    b1 = np.asarray(b1, dtype=np.float32)
    b2 = np.asarray(b2, dtype=np.float32)
    eps_val = float(np.asarray(eps))

    (out_s, pairs_by_window, dstl_tab, call_info, idx_lay, node_at, n_pairs) = _prep_host2(
        np.asarray(edge_index)
    )
    lo_cols = idx_lay["lo"].shape[2]
    hi_cols = idx_lay["hi"].shape[2]
    nc = _build_program2(pairs_by_window, call_info, n_pairs, lo_cols, hi_cols, eps_val)

    import ml_dtypes
    x_lo = np.ascontiguousarray(
        np.concatenate([x[:LO_N], np.zeros((1, D), np.float32)], axis=0).astype(ml_dtypes.bfloat16))
    x_hi = np.ascontiguousarray(
        np.concatenate([x[LO_N:], np.zeros((1, D), np.float32)], axis=0).astype(ml_dtypes.bfloat16))
    b1c = np.ascontiguousarray(b1.reshape(P, 1))
    b2b = np.ascontiguousarray(np.tile(b2.reshape(1, D), (P, 1)))
    iota = np.ascontiguousarray(np.tile(np.arange(P, dtype=np.float32), (P, 1)).astype(ml_dtypes.bfloat16))

    in_maps = []
    for c in range(N_CORES):
        ids = node_at[c].copy()
        zpad = ids < 0
        ids[zpad] = 0
        xt = x[ids].T.copy()
        xt[:, zpad] = 0.0
        in_maps.append(
            {
                "x_lo": x_lo,
                "x_hi": x_hi,
                "xt_rank": np.ascontiguousarray(xt),
                "idx_lo": np.ascontiguousarray(idx_lay["lo"][c]),
                "idx_hi": np.ascontiguousarray(idx_lay["hi"][c]),
                "dstl": np.ascontiguousarray(dstl_tab[c]),
                "ndstl": np.ascontiguousarray(-dstl_tab[c]),
                "iota": iota,
                "w1": W1,
                "w2": W2,
                "b1c": b1c,
                "b2b": b2b,
            }
        )
    res = run_bass_kernel_spmd(nc, in_maps, list(range(N_CORES)))

    out = np.empty((N_NODES, D), dtype=np.float32)
    for c in range(N_CORES):
        rows = np.asarray(res.results[c]["out"])
        ids = node_at[c]
        valid = ids >= 0
        out[ids[valid]] = rows[valid]
    kernel.last_results = res
    return out



# revision 8
# speedup vs baseline: 1.3065x; 1.0226x over previous
"""EpsGINConv TRN2 kernel v2.1: dma_gather + one-hot matmul, bf16 data path.

Structure (same as v2):
  - Gather via GPSIMD dma_gather ucode with x split into lo (<32767) / hi
    halves to fit int16 indices. Edge streams are compacted per window:
    blocks of 128 edges may span window boundaries; such blocks get one
    matmul per touched window with a masked one-hot.
  - Aggregation: psum_hT[f, dst] += Xg_block.T @ OneHot(dstl) — produces h.T
    directly. One-hot built on DVE via tensor_scalar(iota, dstl, is_equal).
  - Self term (1+eps)x added from a host-staged transposed rank-ordered x.

v2.1: gathered x, iota, and one-hots are bf16 (PSUM accumulation stays f32;
MLP stays f32). rel l2 ~1.7e-3 (from bf16 x), well under the 2e-2 gate.

v2.2 (714us -> 380us):
  - num_swdge_queues=4 + queue_num=i%4 on dma_gather: desc-gen runs on all
    four Q7 pairs concurrently (a dispatch whose pair is free retires in
    ~65ns and gathers in background; only same-queue calls serialize).
    Effective ~2.2ns/idx vs 8.5 serial.
  - Deep pools (gather bufs=16/stream, one-hot bufs=40) keep the GPSIMD
    queue primed — a starved dispatch falls back to 8.6us foreground.
  - 1/6 of one-hot builds moved to the idle Scalar engine as
    relu(1 - square(iota - dstl)) (exact for integer labels, including
    dstl=-1 padding), relieving the DVE pacing chain.
"""
import sys

import numpy as np

if "/opt/trn_rl_repo" not in sys.path:
    sys.path.insert(0, "/opt/trn_rl_repo")

import concourse.bass as bass
import concourse.bacc as bacc
import concourse.tile as tile
import concourse.mybir as mybir
from concourse.bass_utils import run_bass_kernel_spmd
from concourse import library_config

P = 128
N_NODES = 50000
D = 128
N_CORES = 8
NPC = N_NODES // N_CORES
NW = (NPC + P - 1) // P
NPAD = NW * P
LO_N = 32767  # x rows [0, 32767) -> lo; row 32767 of x_lo = zeros
HI_ROWS = N_NODES - LO_N + 1  # 17234 incl zero row at 17233
IPC = 1024  # idx per dma_gather call (8 blocks); 8.8ns/idx vs 9.3 at 512, crash boundary is >=1920

F32 = mybir.dt.float32
BF16 = mybir.dt.bfloat16
I16 = mybir.dt.int16
I32 = mybir.dt.int32


def _wrap_idx(calls):
    """calls: list of 1-D int16 arrays (each len%128==0). Returns [128, cols]
    int16 in dma_gather layout: pos j of call -> partition j%16, col j//16,
    replicated 8x down partition groups; plus per-call col offsets."""
    cols = sum(len(a) // 16 for a in calls)
    lay = np.zeros((P, cols), dtype=np.int16)
    offs = []
    o = 0
    for a in calls:
        w = len(a) // 16
        blk = a.reshape(w, 16).T  # [16, w]
        for r in range(8):
            lay[r * 16 : (r + 1) * 16, o : o + w] = blk
        offs.append((o, w, len(a)))
        o += w
    return lay, offs


def _prep_host2(edge_index):
    src = edge_index[0].astype(np.int64)
    dst = edge_index[1].astype(np.int64)

    deg_all = np.bincount(dst, minlength=N_NODES)
    gorder = np.argsort(-deg_all, kind="stable")
    core_of = np.empty(N_NODES, dtype=np.int64)
    crank_of = np.empty(N_NODES, dtype=np.int64)
    grank = np.arange(N_NODES)
    core_of[gorder] = grank % N_CORES
    crank_of[gorder] = grank // N_CORES
    node_at = np.full((N_CORES, NPAD), -1, dtype=np.int64)
    node_at[core_of[gorder], crank_of[gorder]] = gorder

    e_core = core_of[dst]
    e_rank = crank_of[dst]

    per_core = []
    for c in range(N_CORES):
        m = e_core == c
        r_e = e_rank[m]
        s_e = src[m]
        o = np.argsort(r_e, kind="stable")
        per_core.append((r_e[o], s_e[o]))

    # build per-core lo/hi streams; shapes must match across cores -> compute
    # per-stream lengths per core, pad all cores' streams to the max.
    streams = []  # per core: dict(lo=(idx,win,dstl), hi=...)
    for c in range(N_CORES):
        r_s, s_s = per_core[c]
        lo_m = s_s < LO_N
        d = {}
        for name, mm, base in (("lo", lo_m, 0), ("hi", ~lo_m, LO_N)):
            rr = r_s[mm]
            ss = s_s[mm] - base
            d[name] = (ss, rr // P, rr % P)  # idx, window, dstl
        streams.append(d)

    lens = {
        name: max(len(streams[c][name][0]) for c in range(N_CORES))
        for name in ("lo", "hi")
    }
    # pad to common length, then to x128
    out = {}
    for name, zrow in (("lo", LO_N), ("hi", HI_ROWS - 1)):
        L = -(-lens[name] // P) * P
        idx_all = np.full((N_CORES, L), zrow, dtype=np.int64)
        win_all = np.full((N_CORES, L), -1, dtype=np.int64)
        dstl_all = np.full((N_CORES, L), -1, dtype=np.int64)
        for c in range(N_CORES):
            ii, ww, dd = streams[c][name]
            n = len(ii)
            idx_all[c, :n] = ii
            win_all[c, :n] = ww
            dstl_all[c, :n] = dd
        out[name] = (idx_all.astype(np.int16), win_all, dstl_all, L)

    # pairs: per stream, per block, touched windows (order by core-0's... must
    # be identical across cores! windows touched by block b can differ per
    # core. Use the UNION across cores so the program is common.
    pair_list = []  # (stream, block, window) in program order grouping by window
    pairs_by_window = [[] for _ in range(NW)]
    dstl_cols = []  # list of (stream, block, window) -> one column per pair
    for sname in ("lo", "hi"):
        idx_all, win_all, dstl_all, L = out[sname]
        nb = L // P
        for b in range(nb):
            ws = set()
            for c in range(N_CORES):
                ws |= set(np.unique(win_all[:, b * P : (b + 1) * P]).tolist())
            ws.discard(-1)
            for w in sorted(ws):
                pid = len(dstl_cols)
                dstl_cols.append((sname, b, w))
                pairs_by_window[w].append((sname, b, pid))

    n_pairs = len(dstl_cols)
    dstl_tab = np.full((N_CORES, P, n_pairs), -1.0, dtype=np.float32)
    for pid, (sname, b, w) in enumerate(dstl_cols):
        _, win_all, dstl_all, _ = out[sname]
        for c in range(N_CORES):
            wv = win_all[c, b * P : (b + 1) * P]
            dv = dstl_all[c, b * P : (b + 1) * P]
            sel = wv == w
            dstl_tab[c, sel, pid] = dv[sel]

    # calls: groups of IPC idx per stream
    call_info = {}
    idx_lay = {}
    for sname in ("lo", "hi"):
        idx_all, _, _, L = out[sname]
        calls = []
        spans = []
        o = 0
        while o < L:
            n = min(IPC, L - o)
            spans.append((o // P, n // P, n))  # first block, nblocks, nidx
            o += n
        call_info[sname] = spans
        lays = []
        for c in range(N_CORES):
            arrs = [idx_all[c, s * P : s * P + n] for (s, _, n) in spans]
            lay, offs = _wrap_idx(arrs)
            lays.append(lay)
            call_info[sname + "_offs"] = offs
        idx_lay[sname] = np.stack(lays)

    return out, pairs_by_window, dstl_tab, call_info, idx_lay, node_at, n_pairs


def _build_program2(pairs_by_window, call_info, n_pairs, lo_cols, hi_cols, eps_val):
    nc = bacc.Bacc("TRN2", target_bir_lowering=False, debug=False, num_devices=N_CORES,
                   num_swdge_queues=4)
    xlo_d = nc.dram_tensor("x_lo", [LO_N + 1, D], BF16, kind="ExternalInput").ap()
    xhi_d = nc.dram_tensor("x_hi", [HI_ROWS, D], BF16, kind="ExternalInput").ap()
    xt_d = nc.dram_tensor("xt_rank", [P, NPAD], F32, kind="ExternalInput").ap()
    ilo_d = nc.dram_tensor("idx_lo", [P, lo_cols], I16, kind="ExternalInput").ap()
    ihi_d = nc.dram_tensor("idx_hi", [P, hi_cols], I16, kind="ExternalInput").ap()
    dstl_d = nc.dram_tensor("dstl", [P, n_pairs], F32, kind="ExternalInput").ap()
    ndstl_d = nc.dram_tensor("ndstl", [P, n_pairs], F32, kind="ExternalInput").ap()
    iota_d = nc.dram_tensor("iota", [P, P], BF16, kind="ExternalInput").ap()
    w1_d = nc.dram_tensor("w1", [D, D], F32, kind="ExternalInput").ap()
    w2_d = nc.dram_tensor("w2", [D, D], F32, kind="ExternalInput").ap()
    b1_d = nc.dram_tensor("b1c", [P, 1], F32, kind="ExternalInput").ap()
    b2_d = nc.dram_tensor("b2b", [P, D], F32, kind="ExternalInput").ap()
    out_d = nc.dram_tensor("out", [NPAD, D], F32, kind="ExternalOutput").ap()

    Relu = mybir.ActivationFunctionType.Relu
    Square = mybir.ActivationFunctionType.Square
    IsEq = mybir.AluOpType.is_equal
    srcs = {"lo": xlo_d, "hi": xhi_d}
    idxs_d = {"lo": ilo_d, "hi": ihi_d}

    with tile.TileContext(nc) as tc:
        with (
            tc.tile_pool(name="const", bufs=1) as cp,
            tc.tile_pool(name="glo", bufs=24) as glo_p,
            tc.tile_pool(name="ghi", bufs=24) as ghi_p,
            tc.tile_pool(name="oh", bufs=40) as ohp,
            tc.tile_pool(name="t2p", bufs=8) as t2p,
            tc.tile_pool(name="hbuf", bufs=6) as hpool,
            tc.tile_pool(name="obuf", bufs=4) as opool,
            tc.tile_pool(name="pht", bufs=4, space="PSUM") as phtp,
            tc.tile_pool(name="pz", bufs=2, space="PSUM") as pzp,
            tc.tile_pool(name="po", bufs=2, space="PSUM") as pop,
        ):
            with tc.tile_critical():
                nc.gpsimd.load_library(library_config.mlp)
            ilo_t = cp.tile([P, lo_cols], I16)
            ihi_t = cp.tile([P, hi_cols], I16)
            NCH = 3
            for lo, hi in [(i * lo_cols // NCH, (i + 1) * lo_cols // NCH) for i in range(NCH)]:
                nc.sync.dma_start(ilo_t[:, lo:hi], ilo_d[:, lo:hi])
            for lo, hi in [(i * hi_cols // NCH, (i + 1) * hi_cols // NCH) for i in range(NCH)]:
                nc.sync.dma_start(ihi_t[:, lo:hi], ihi_d[:, lo:hi])
            dstl_t = cp.tile([P, n_pairs], F32)
            nc.sync.dma_start(dstl_t[:], dstl_d[:])
            ndstl_t = cp.tile([P, n_pairs], F32)
            nc.sync.dma_start(ndstl_t[:], ndstl_d[:])
            iota_t = cp.tile([P, P], BF16)
            nc.sync.dma_start(iota_t[:], iota_d[:])
            xt_t = cp.tile([P, NPAD], F32)
            nc.sync.dma_start(xt_t[:], xt_d[:])
            w1_t = cp.tile([D, D], F32)
            nc.sync.dma_start(w1_t[:], w1_d[:])
            w2_t = cp.tile([D, D], F32)
            nc.sync.dma_start(w2_t[:], w2_d[:])
            b1_t = cp.tile([P, 1], F32)
            nc.sync.dma_start(b1_t[:], b1_d[:])
            b2_t = cp.tile([P, D], F32)
            nc.sync.dma_start(b2_t[:], b2_d[:])

            gp = {"lo": glo_p, "hi": ghi_p}
            g_tiles = {"lo": {}, "hi": {}}
            issued = {"lo": set(), "hi": set()}
            qctr = [0]

            def ensure_call(sname, ci):
                if ci in issued[sname]:
                    return
                issued[sname].add(ci)
                b0, nb, nidx = call_info[sname][ci]
                off, wcols, _ = call_info[sname + "_offs"][ci]
                g = gp[sname].tile([P, nb, D], BF16, tag="g")
                nc.gpsimd.dma_gather(
                    g[:],
                    srcs[sname][:],
                    idxs_d and (ilo_t if sname == "lo" else ihi_t)[:, off : off + wcols],
                    nidx,
                    nidx,
                    D,
                    queue_num=qctr[0] % 4,
                )
                qctr[0] += 1
                g_tiles[sname][ci] = g

            for w in range(NW):
                pairs = pairs_by_window[w]
                psum_ht = phtp.tile([P, D], F32, tag="pht")
                for i, (sname, b, pid) in enumerate(pairs):
                    ci = b // (IPC // P)
                    ensure_call(sname, ci)
                    b0, nb, _ = call_info[sname][ci]
                    g = g_tiles[sname][ci]
                    oh = ohp.tile([P, P], BF16, tag="oh")
                    if pid % 6 == 5:
                        t2 = t2p.tile([P, P], F32, tag="t2")
                        nc.scalar.activation(
                            t2[:], iota_t[:], Square, bias=ndstl_t[:, pid : pid + 1]
                        )
                        nc.scalar.activation(oh[:], t2[:], Relu, bias=1.0, scale=-1.0)
                    else:
                        nc.vector.tensor_scalar(
                            oh[:], iota_t[:], dstl_t[:, pid : pid + 1], None, IsEq
                        )
                    nc.tensor.matmul(
                        psum_ht[:],
                        lhsT=g[:, b - b0, :],
                        rhs=oh[:],
                        start=(i == 0),
                        stop=(i == len(pairs) - 1),
                    )
                ht_sb = hpool.tile([P, D], F32, tag="ht")
                xt_win = xt_t[:, w * P : (w + 1) * P]
                if not pairs:
                    if eps_val != 0.0:
                        nc.scalar.mul(ht_sb[:], xt_win, float(1.0 + eps_val))
                    else:
                        nc.vector.tensor_copy(ht_sb[:], xt_win)
                elif eps_val != 0.0:
                    xs_t = hpool.tile([P, D], F32, tag="xs")
                    nc.scalar.mul(xs_t[:], xt_win, float(1.0 + eps_val))
                    nc.vector.tensor_add(ht_sb[:], xs_t[:], psum_ht[:])
                else:
                    nc.vector.tensor_add(ht_sb[:], xt_win, psum_ht[:])
                psum_z = pzp.tile([P, D], F32, tag="pz")
                nc.tensor.matmul(psum_z[:], lhsT=w1_t[:], rhs=ht_sb[:], start=True, stop=True)
                zt_sb = hpool.tile([P, D], F32, tag="zt")
                nc.scalar.activation(zt_sb[:], psum_z[:], Relu, bias=b1_t[:, :1])
                psum_o = pop.tile([P, D], F32, tag="po")
                nc.tensor.matmul(psum_o[:], lhsT=zt_sb[:], rhs=w2_t[:], start=True, stop=True)
                o_sb = opool.tile([P, D], F32, tag="o")
                nc.vector.tensor_add(o_sb[:], psum_o[:], b2_t[:])
                o2_sb = opool.tile([P, D], F32, tag="o2")
                nc.scalar.activation(o2_sb[:], o_sb[:], Relu)
                nc.sync.dma_start(out_d[w * P : (w + 1) * P, :], o2_sb[:])
    nc.compile()
    return nc


def kernel(x, edge_index, W1, b1, W2, b2, eps):
    x = np.ascontiguousarray(np.asarray(x, dtype=np.float32))
    W1 = np.asarray(W1, dtype=np.float32)
    W2 = np.asarray(W2, dtype=np.float32)
    b1 = np.asarray(b1, dtype=np.float32)
    b2 = np.asarray(b2, dtype=np.float32)
    eps_val = float(np.asarray(eps))

    (out_s, pairs_by_window, dstl_tab, call_info, idx_lay, node_at, n_pairs) = _prep_host2(
        np.asarray(edge_index)
    )
    lo_cols = idx_lay["lo"].shape[2]
    hi_cols = idx_lay["hi"].shape[2]
    nc = _build_program2(pairs_by_window, call_info, n_pairs, lo_cols, hi_cols, eps_val)

    import ml_dtypes
    x_lo = np.ascontiguousarray(
        np.concatenate([x[:LO_N], np.zeros((1, D), np.float32)], axis=0).astype(ml_dtypes.bfloat16))
    x_hi = np.ascontiguousarray(
        np.concatenate([x[LO_N:], np.zeros((1, D), np.float32)], axis=0).astype(ml_dtypes.bfloat16))
    b1c = np.ascontiguousarray(b1.reshape(P, 1))
    b2b = np.ascontiguousarray(np.tile(b2.reshape(1, D), (P, 1)))
    iota = np.ascontiguousarray(np.tile(np.arange(P, dtype=np.float32), (P, 1)).astype(ml_dtypes.bfloat16))

    in_maps = []
    for c in range(N_CORES):
        ids = node_at[c].copy()
        zpad = ids < 0
        ids[zpad] = 0
        xt = x[ids].T.copy()
        xt[:, zpad] = 0.0
        in_maps.append(
            {
                "x_lo": x_lo,
                "x_hi": x_hi,
                "xt_rank": np.ascontiguousarray(xt),
                "idx_lo": np.ascontiguousarray(idx_lay["lo"][c]),
                "idx_hi": np.ascontiguousarray(idx_lay["hi"][c]),
                "dstl": np.ascontiguousarray(dstl_tab[c]),
                "ndstl": np.ascontiguousarray(-dstl_tab[c]),
                "iota": iota,
                "w1": W1,
                "w2": W2,
                "b1c": b1c,
                "b2b": b2b,
            }
        )
    res = run_bass_kernel_spmd(nc, in_maps, list(range(N_CORES)))

    out = np.empty((N_NODES, D), dtype=np.float32)
    for c in range(N_CORES):
        rows = np.asarray(res.results[c]["out"])
        ids = node_at[c]
        valid = ids >= 0
        out[ids[valid]] = rows[valid]
    kernel.last_results = res
    return out

